# revision 23
# baseline (speedup 1.0000x reference)
"""Causal self-attention (B=4, T=2048, C=1024, H=16) on 8 NeuronCores.

Sharding: data-parallel over batch (4) x tensor-parallel over heads (2 groups
of 8 heads) = 8 cores. Each core computes QKV for its 8 heads, causal
flash-style attention, and a partial output projection (row-parallel).
Host sums the two partial projections per batch and adds b_proj.

All matmul operands are stored fp16 (the PE multiplies at ~fp22 internally,
so fp16's 11-bit mantissa matches fp32r precision while halving memory and
enabling hardware DMA-transpose + fast weight loads). All accumulation is
fp32 in PSUM.

DMA layout: x^T is produced by ONE hardware DMA-transpose per 512-row chunk
into a [128, 8, 512] SBUF tile (block kk holds channels kk*128+p, the same
layout as per-block transposes). Weight/const tensors are packed into few
large DMAs (HWDGE issue costs ~625ns per DMA instruction, so instruction
count matters more than bytes), issued across the SP/ACT/Pool queues since
an issuing sequencer is held for its DMA's duration.

Per-core device kernel (Bass/Tile):
  phase 1: per 512-row chunk: q^T,k^T [ch,T] via fp16 matmuls emitted
           kk-inner over m-pairs (so the first matmul only needs the first
           weight slice to have landed); v [T,ch] in 65-col blocks with a
           ones column that makes the PV matmul emit softmax denominators.
           qk bias fused into the PSUM->SBUF copy, v bias as a K=1 matmul.
  phase 2: per (head, 512-wide tq chunk): scores^T = k^T.T @ q^T in PSUM,
           exp on ACT (scale=1/8) -> P^T fp16, causal handled by restricting
           diagonal-block columns + affine_select zero-fill, PV accumulate
           y^T[65,512] where row 64 = softmax denominator l. Normalization:
           r=1/l on a [1,512] tile, broadcast to [64,512] with a K=1 PE
           matmul, applied on DVE straight from PSUM.
           Phase-1 chunks for later tq are interleaved into this stream so
           the ACT-bound exp work overlaps PE-bound qkv matmuls.
  phase 3 (interleaved per tq chunk): out = y^T.T @ w_proj, staged per-chunk
           into one [128, 4, 1024] fp16 tile, ONE output DMA per chunk.
"""

from contextlib import nullcontext

import numpy as np

import concourse.bass as bass
import concourse.mybir as mybir
from concourse import bacc
from concourse.tile import TileContext
from concourse.bass_utils import run_bass_kernel_spmd

B, T, C, H, D = 4, 2048, 1024, 16, 64
CQ = 512          # q (or k or v) channels per core = 8 heads * 64
HPC = 8           # heads per core
F32 = mybir.dt.float32
F16 = mybir.dt.float16
Exp = mybir.ActivationFunctionType.Exp
is_ge = mybir.AluOpType.is_ge

TCH = 512         # phase-1 T-chunk
NCH = T // TCH    # 4 chunks
VSTR = HPC * (D + 1)   # 520: v_ext per-T-block stride (8 heads x 65)
C16W = 8 * 512 + 512 + 128   # packed fp16 consts: w_v | b_vz row | ones row


def build_nc(loop_n=1):
    """loop_n > 1 wraps the whole kernel in a device-side repeat loop
    (benchmarking only -- output is identical every iteration)."""
    nc = bacc.Bacc("TRN2", target_bir_lowering=False, debug=False, num_devices=8)

    x = nc.dram_tensor("x", [T, C], F16, kind="ExternalInput")
    w_qk = nc.dram_tensor("w_qk", [C, 2 * CQ], F16, kind="ExternalInput")
    b_qk = nc.dram_tensor("b_qk", [1, 2 * CQ], F32, kind="ExternalInput")
    c16 = nc.dram_tensor("c16", [128, C16W], F16, kind="ExternalInput")
    w_pj = nc.dram_tensor("w_pj", [CQ, C], F16, kind="ExternalInput")
    out = nc.dram_tensor("out", [T, C], F16, kind="ExternalOutput")

    with TileContext(nc) as tc:
        with (
            tc.tile_pool(name="const", bufs=1) as pc,
            tc.tile_pool(name="persist", bufs=1) as pp,
            tc.tile_pool(name="work", bufs=2) as pw,
            tc.tile_pool(name="psum", bufs=2, space="PSUM") as ps,
            tc.For_i(0, loop_n, 1) if loop_n > 1 else nullcontext(),
        ):
            # ---- persistent activations (declared first: memsets are
            # engine-local and overlap the input DMA stream) ----
            qT = [pp.tile([128, T], F16, name=f"qT{m}") for m in range(4)]
            # per-head k^T, zero-padded to K=128 so the scores matmul streams
            # the full qT tile at full SBUF bandwidth (the zero half
            # multiplies the sibling head's rows away)
            kZ = [pp.tile([128, T], F16, name=f"kZ{i}") for i in range(HPC)]
            for i in range(HPC):
                z0 = 64 * (1 - i % 2)
                nc.gpsimd.memset(kZ[i][z0:z0 + 64, :], 0.0)
            yT = [pp.tile([128, T], F16, name=f"yT{m}") for m in range(4)]
            v_ext = pp.tile([128, (T // 128) * VSTR], F16, name="v_ext")
            v_ones = v_ext[:].rearrange(
                "p (t i d) -> p t i d", i=HPC, d=D + 1
            )[:, :, :, D:D + 1]
            nc.gpsimd.memset(v_ones, 1.0)

            # ---- constants / weights: few large DMAs, ordered so the first
            # qk matmul's operands land first ----
            w_qk_sb = pc.tile([128, 8 * 1024], F16, name="w_qk_sb")
            c16_sb = pc.tile([128, C16W], F16, name="c16_sb")
            w_v_sb = c16_sb[:, 0:8 * 512]
            b_vz_sb = c16_sb[:, 8 * 512:8 * 512 + 512]
            onesz_sb = c16_sb[:, 8 * 512 + 512:]
            b_qk2 = pc.tile([128, 8], F32, name="b_qk2")

            xT_tiles = {}

            def xload(ct, k0=0, kn=8):
                """One hardware DMA-transpose for channel blocks [k0, k0+kn)
                of chunk ct (SBUF block kk holds channels kk*128+p)."""
                if ct not in xT_tiles:
                    xT_tiles[ct] = pw.tile(
                        [128, 8 * TCH], F16, name="xT_c", tag="xT_c", bufs=2)
                xT_c = xT_tiles[ct]
                T0 = ct * TCH
                nc.sync.dma_start_transpose(
                    xT_c[:].rearrange("p (k t) -> p k t", k=8)[:, k0:k0 + kn, :],
                    x[T0:T0 + TCH, k0 * 128:(k0 + kn) * 128],
                )
                return xT_c

            # weights issue from the ACT queue in parallel with the SP-queue
            # transposes (an issuing sequencer is held for its DMA's
            # duration, so same-queue DMAs serialize); chunk 0 is split by
            # channel halves so the first qk matmuls (kk 0-3) start early
            xload(0, 0, 4)
            xload(0, 4, 4)
            nc.scalar.dma_start(
                out=w_qk_sb[:, 0:4 * 1024].rearrange("p (k j) -> p k j", k=4),
                in_=w_qk[0:512, :].rearrange("(k p) j -> p k j", p=128))
            nc.scalar.dma_start(
                out=w_qk_sb[:, 4 * 1024:].rearrange("p (k j) -> p k j", k=4),
                in_=w_qk[512:1024, :].rearrange("(k p) j -> p k j", p=128))
            nc.gpsimd.dma_start(out=b_qk2[:], in_=b_qk[0, :].rearrange(
                "(m p) -> p m", p=128))
            nc.scalar.dma_start(out=c16_sb[:], in_=c16[:])
            xload(1)

            def qk(xT_c, ct, g):
                # q,k out^T layout [ch, T-chunk]; m-pair (2g, 2g+1) with kk
                # inner; bias fused into the PSUM->SBUF copy
                T0 = ct * TCH
                pair = []
                for m in (2 * g, 2 * g + 1):
                    qk_ps = ps.tile([128, TCH], F32, name="qk_ps", tag="mm", bufs=2)
                    pair.append((m, qk_ps))
                for kk in range(8):
                    for m, qk_ps in pair:
                        nc.tensor.matmul(
                            qk_ps[:],
                            w_qk_sb[:, kk * 1024 + m * 128:kk * 1024 + (m + 1) * 128],
                            xT_c[:, kk * TCH:(kk + 1) * TCH],
                            start=(kk == 0),
                            stop=(kk == 7),
                        )
                for m, qk_ps in pair:
                    if m < 4:
                        nc.vector.tensor_scalar_add(
                            qT[m][:, T0:T0 + TCH], qk_ps[:], b_qk2[:, m:m + 1]
                        )
                    else:
                        for half in range(2):
                            ih = 2 * (m - 4) + half
                            rows = slice(64 * half, 64 * half + 64)
                            nc.vector.tensor_scalar_add(
                                kZ[ih][rows, T0:T0 + TCH],
                                qk_ps[rows, :],
                                b_qk2[rows, m:m + 1],
                            )

            def vpart(xT_c, ct, tt):
                # v: natural layout [T-block, ch], interleaved into v_ext
                v_ps = ps.tile([128, CQ], F32, name="v_ps", tag="mm", bufs=2)
                for kk in range(8):
                    nc.tensor.matmul(
                        v_ps[:],
                        xT_c[:, kk * TCH + tt * 128:kk * TCH + (tt + 1) * 128],
                        w_v_sb[:, kk * 512:(kk + 1) * 512],
                        start=(kk == 0),
                        stop=False,
                    )
                # bias broadcast as a K=1 matmul (ones row x bias row)
                nc.tensor.matmul(
                    v_ps[:],
                    onesz_sb[0:1, 0:128],
                    b_vz_sb[0:1, :],
                    start=False,
                    stop=True,
                )
                tb = ct * (TCH // 128) + tt
                dst = v_ext[:, tb * VSTR:(tb + 1) * VSTR].rearrange(
                    "p (i d) -> p i d", d=D + 1
                )[:, :, 0:D]
                src = v_ps[:].rearrange("p (i d) -> p i d", d=D)
                nc.vector.tensor_copy(dst, src)

            def phase1_chunk_steps(ct, skip_load=False):
                """Yield emission closures for one phase-1 chunk, so chunks
                can be interleaved into the attention stream."""
                box = {}

                def load():
                    box['xT'] = xT_tiles[ct] if skip_load else xload(ct)

                yield load
                for g in range(4):
                    yield lambda g=g: qk(box['xT'], ct, g)
                for t0 in range(0, TCH // 128, 2):
                    yield lambda t0=t0: (vpart(box['xT'], ct, t0),
                                         vpart(box['xT'], ct, t0 + 1))

            def phase1_chunk(ct, skip_load=False):
                for step in phase1_chunk_steps(ct, skip_load):
                    step()

            # global PV software pipeline: PV matmuls trail their exp by >=2
            # groups and drain while the NEXT head's scores stream, so the
            # in-order PE never synchronously waits on ACT/Pool at head ends
            pv_pending = []  # [(P, y_ps, i, nblk, [(tkb, oc0, pc0, w), ...])]

            def pv_flush(depth=0):
                while len(pv_pending) > depth:
                    P, y_ps, i, nblk, items = pv_pending.pop(0)
                    for tkb, oc0, pc0, w in items:
                        nc.tensor.matmul(
                            y_ps[:, oc0:oc0 + w],
                            v_ext[:, tkb * VSTR + i * (D + 1):
                                  tkb * VSTR + (i + 1) * (D + 1)],
                            P[:, pc0:pc0 + w],
                            start=(tkb == 0),
                            stop=(tkb == nblk - 1),
                            skip_group_check=True,
                        )

            def attention(c, i):
                """Emit scores+exp groups; PV is emitted via the global
                pipeline. Returns (y_ps, m, p0, c) for deferred
                normalization."""
                m = i // 2
                p0 = 64 * (i % 2)
                nblk = 4 * c + 4
                y_ps = ps.tile([D + 1, 512], F32, name="y_ps", tag="psy", bufs=2)

                def group(items):
                    """One psum tile + one exp over several blocks.
                    items: (tkb, out_col0, p_col0, width, straddler)."""
                    total = items[-1][2] + items[-1][3]
                    s_g = ps.tile([128, 1024], F32, name="s_g", tag="sg", bufs=2)
                    P_g = pw.tile([128, 1024], F16, name="P_g", tag="P_t", bufs=8)
                    for tkb, oc0, pc0, w, _ in items:
                        nc.tensor.matmul(
                            s_g[:, pc0:pc0 + w],
                            kZ[i][:, tkb * 128:(tkb + 1) * 128],
                            qT[m][:, c * 512 + oc0:(c + 1) * 512],
                            start=True,
                            stop=True,
                        )
                    nc.scalar.activation(
                        P_g[:, 0:total], s_g[:, 0:total], Exp, scale=0.125)
                    for tkb, oc0, pc0, w, straddler in items:
                        if straddler:
                            # keep where (piece-local y) >= x
                            nc.gpsimd.affine_select(
                                out=P_g[:, pc0:pc0 + w],
                                in_=P_g[:, pc0:pc0 + w],
                                compare_op=is_ge,
                                fill=0.0,
                                base=0,
                                pattern=[[1, w]],
                                channel_multiplier=-1,
                            )
                    pv_flush(depth=2)
                    pv_pending.append((P_g, y_ps, i, nblk,
                                       [it[:4] for it in items]))

                # full (below-diagonal) blocks in pairs; diagonal straddlers
                # packed j0+j1 and j2+j3 to amortize ACT fixed cost
                for pair in range(2 * c):
                    group([(2 * pair, 0, 0, 512, False),
                           (2 * pair + 1, 0, 512, 512, False)])
                group([(4 * c, 0, 0, 512, True),
                       (4 * c + 1, 128, 512, 384, True)])
                group([(4 * c + 2, 256, 0, 256, True),
                       (4 * c + 3, 384, 256, 128, True)])
                return (y_ps, m, p0, c)

            def normalize(pend):
                y_ps, m, p0, c = pend
                r_row = pw.tile([1, 512], F16, name="r_row", tag="r_row", bufs=2)
                with nc.allow_low_precision(reason="fp16 matches PE fp22 input precision"):
                    nc.vector.reciprocal(r_row[0:1, :], y_ps[D:D + 1, :])
                # broadcast r to 64 partitions on the Pool engine
                R_sb = pw.tile([64, 512], F16, name="R_sb", tag="R_sb", bufs=2)
                nc.gpsimd.partition_broadcast(R_sb[:], r_row[0:1, :])
                with nc.allow_low_precision(reason="fp16 matches PE fp22 input precision"):
                    nc.vector.tensor_mul(
                        yT[m][p0:p0 + 64, c * 512:(c + 1) * 512],
                        y_ps[0:D, :],
                        R_sb[:],
                    )

            o_tiles = {}

            def proj(c, mt):
                if mt % 4 == 0:
                    o_tiles[c] = pw.tile([128, 4 * 1024], F16, name="o_big",
                                         tag="o_big", bufs=2)
                o_big = o_tiles[c]
                for nn in range(2):
                    pj_ps = ps.tile([128, 512], F32, name="pj_ps", tag="mm", bufs=2)
                    for kk in range(4):
                        nc.tensor.matmul(
                            pj_ps[:],
                            yT[kk][:, mt * 128:(mt + 1) * 128],
                            w_pj_sb[:, kk * 1024 + nn * 512:kk * 1024 + (nn + 1) * 512],
                            start=(kk == 0),
                            stop=(kk == 3),
                        )
                    with nc.allow_low_precision(reason="fp16 out matches tolerance"):
                        nc.vector.tensor_copy(
                            o_big[:, (mt % 4) * 1024 + nn * 512:
                                  (mt % 4) * 1024 + (nn + 1) * 512], pj_ps[:])

            def proj_out(c):
                nc.gpsimd.dma_start(
                    out=out[c * 512:(c + 1) * 512, :].rearrange(
                        "(mt p) j -> p mt j", p=128),
                    in_=o_tiles.pop(c)[:].rearrange("p (mt j) -> p mt j", mt=4),
                )

            # ---- emission: chunks 0-1 up front (attention c=0 needs them),
            # remaining phase-1 chunks interleaved into the attention stream
            # so ACT-bound exp work overlaps PE-bound qkv matmuls.
            phase1_chunk(0, skip_load=True)
            phase1_chunk(1, skip_load=True)

            w_pj_sb = pc.tile([128, 4 * 1024], F16, name="w_pj_sb")
            nc.scalar.dma_start(
                out=w_pj_sb[:].rearrange("p (k j) -> p k j", k=4),
                in_=w_pj[:].rearrange("(k p) j -> p k j", p=128))

            p1_queue = []
            for ct in range(2, NCH):
                p1_queue.extend(phase1_chunk_steps(ct))

            # normalization of head i emitted after head i+1's blocks so the
            # PE never waits on the DVE reciprocal chain
            pend = None
            proj_q = []
            for c in range(T // 512):
                for i in range(HPC):
                    nxt = attention(c, i)
                    # drain any previous head's PV leftovers so its deferred
                    # normalize below only reads completed y_ps
                    pv_flush(2)
                    if pend is not None:
                        normalize(pend)
                    pend = nxt
                    # later phase-1 chunks emitted during attention chunk c
                    # (needed by attention chunk c+1)
                    if p1_queue and c < 3:
                        p1_queue.pop(0)()
                    # previous chunk's projection spread over this chunk's heads
                    if proj_q:
                        proj_q.pop(0)()
                pv_flush(0)
                normalize(pend)
                pend = None
                proj_q = [lambda mt=mt, c=c: proj(c, mt)
                          for mt in range(4 * c, 4 * c + 4)]
                proj_q.append(lambda c=c: proj_out(c))
            for f in proj_q:
                f()

    nc.compile()
    return nc


_NC = None


def _get_nc():
    global _NC
    if _NC is None:
        _NC = build_nc()
    return _NC


def make_in_maps(x, w_attn, b_attn, w_proj):
    x = np.asarray(x, dtype=np.float32)
    w_attn = np.asarray(w_attn, dtype=np.float32)
    b_attn = np.asarray(b_attn, dtype=np.float32)
    w_proj = np.asarray(w_proj, dtype=np.float32)
    in_maps = []
    for core in range(8):
        b, g = divmod(core, 2)
        s = g * CQ
        w_v_nat = w_attn[:, 2 * C + s:2 * C + s + CQ]
        c16 = np.zeros((128, C16W), np.float32)
        # SBUF block kk holds contraction rows kk*128+p (transpose layout)
        c16[:, 0:8 * 512] = w_v_nat.reshape(8, 128, 512).transpose(1, 0, 2).reshape(128, 4096)
        c16[0, 8 * 512:8 * 512 + 512] = b_attn[2 * C + s:2 * C + s + CQ]
        c16[0, 8 * 512 + 512:] = 1.0
        in_maps.append({
            "x": np.ascontiguousarray(x[b]).astype(np.float16),
            "w_qk": np.ascontiguousarray(
                np.concatenate([w_attn[:, s:s + CQ], w_attn[:, C + s:C + s + CQ]],
                               axis=1)).astype(np.float16),
            "b_qk": np.concatenate(
                [b_attn[s:s + CQ], b_attn[C + s:C + s + CQ]]
            ).reshape(1, 2 * CQ).astype(np.float32),
            "c16": np.ascontiguousarray(c16).astype(np.float16),
            "w_pj": np.ascontiguousarray(w_proj[s:s + CQ, :]).astype(np.float16),
        })
    return in_maps


def kernel(x, w_attn, b_attn, w_proj, b_proj):
    nc = _get_nc()
    in_maps = make_in_maps(x, w_attn, b_attn, w_proj)
    res = run_bass_kernel_spmd(nc, in_maps, list(range(8)))
    b_proj = np.asarray(b_proj, dtype=np.float32)
    out = np.empty((B, T, C), dtype=np.float32)
    for b in range(B):
        out[b] = (res.results[2 * b]["out"].astype(np.float32)
                  + res.results[2 * b + 1]["out"].astype(np.float32) + b_proj)
    return out
